# revision 5
# baseline (speedup 1.0000x reference)
"""BoxCrop kernel for Trainium2 (8 NeuronCores, Bass/Tile).

Fused crop -> aspect-preserving bilinear resize (long side 336) -> square pad
(fill=127) for a batch of 64 images [64,3,768,768] with per-image XYWH boxes.

Strategy (pure data-parallel, 8 images per core):
- Host computes, per image: crop-row gather offsets (128-element-block
  indices into the image shard viewed as [n_blocks, 128]; each descriptor
  reads 512 contiguous elements = 4 blocks covering image columns
  [128*(xb//128), +512) which always contains the crop columns), clamped
  crop-local source coordinates (syc for rows; sxc shifted by the xb%128
  residual for columns; -1e6 for pad rows/cols), and pad-fill params.
- Device per image-channel:
    crop[r, 0:512] via indirect-DMA row gather (f32r, 512B+ descriptors);
    A'[r,i]  = -relu(1-|r-syc_i|)   r in [0,384)   (3 chunks)
    Wx'[k,j] = -relu(1-|k-sxc_j|)   k in [0,512)   (4 chunks)
    (negated tents == exact bilinear weights incl. boundary clamping;
     the two negations cancel in the product)
    RT = crop^T @ A'  (f32r matmuls, contraction over r, partition = k);
    M  = RT-contraction with Wx' (over k) accumulated in PSUM;
    out = M + (127 - 127*vy_i*vx_j)  (pad fill) during PSUM->SBUF.
"""
import numpy as np

import concourse.bacc as bacc
import concourse.bass as bass
import concourse.tile as tile
from concourse import mybir
from concourse.bass import AP, IndirectOffsetOnAxis
from concourse.bass_utils import run_bass_kernel_spmd

F32 = mybir.dt.float32
F32R = mybir.dt.float32r
I32 = mybir.dt.int32

N_CORES = 8
B = 64
BL = B // N_CORES          # images per core
C = 3
H = W = 768
O = 336                    # output size
RROWS = 384                # gathered crop rows (static max)
KCOLS = 512                # gathered columns per row (4 x 128-elem blocks)
PLANE = H * W
IMG_ELEMS = C * PLANE
TOT_ELEMS = BL * IMG_ELEMS
BLK = 128
N_BLOCKS = TOT_ELEMS // BLK
ROW_BLKS = W // BLK        # 6 blocks per image row
BOUND = N_BLOCKS - KCOLS // BLK   # max valid gather start block
BIG = 1 << 27
FILL = 127.0
CROP_BUFS = 3              # first CROP_BUFS images must fully write their tile

_CACHED = None
LAST_RESULT = None


def _build(reps: int = 1):
    nc = bacc.Bacc("TRN2", target_bir_lowering=False, debug=False)

    imgs = nc.dram_tensor("imgs", [N_BLOCKS, BLK], F32R, kind="ExternalInput")
    offs = nc.dram_tensor("offs", [128, BL * 9], I32, kind="ExternalInput")
    par_row = nc.dram_tensor("par_row", [128, BL * 1008], F32, kind="ExternalInput")
    par_col = nc.dram_tensor("par_col", [128, BL * 3], F32, kind="ExternalInput")
    iota = nc.dram_tensor("iota", [128, 4], F32, kind="ExternalInput")
    out = nc.dram_tensor("out", [BL, C, O, O], F32, kind="ExternalOutput")

    with tile.TileContext(nc) as tc:
        with (
            tc.tile_pool(name="const", bufs=1) as cpool,
            tc.tile_pool(name="crop", bufs=CROP_BUFS) as crop_pool,
            tc.tile_pool(name="tent", bufs=2) as tent_pool,
            tc.tile_pool(name="dtmp", bufs=3) as dtmp_pool,
            tc.tile_pool(name="fill", bufs=2) as fill_pool,
            tc.tile_pool(name="rt", bufs=3) as rt_pool,
            tc.tile_pool(name="osb", bufs=2) as out_pool,
            tc.tile_pool(name="ps1", bufs=4, space="PSUM") as ps1,
            tc.tile_pool(name="ps2", bufs=4, space="PSUM") as ps2,
        ):
            offs_sb = cpool.tile([128, BL * 9], I32, tag="offs")
            nc.sync.dma_start(offs_sb[:], offs[:])
            par_sb = cpool.tile([128, BL * 1008], F32, tag="par")
            nc.sync.dma_start(par_sb[:], par_row[:])
            parcol_sb = cpool.tile([128, BL * 3], F32, tag="parcol")
            nc.sync.dma_start(parcol_sb[:], par_col[:])
            iota_sb = cpool.tile([128, 4], F32, tag="iota")
            nc.sync.dma_start(iota_sb[:], iota[:])

            for b in range(reps * BL):
                b = b % BL
                bc = par_sb[:, b * 1008 : (b + 1) * 1008]

                # negated tents, f32r. tent slots: 2t   = A' chunk t (t<3)
                #                                  2t+1 = Wx' chunk t (t<3)
                #                                  6    = Wx' chunk 3
                tent = tent_pool.tile([128, 7, 336], F32R, tag="tent")
                for t in range(3):
                    dtmp = dtmp_pool.tile([128, 672], F32, tag="dtmp")
                    nc.scalar.activation(
                        dtmp[:],
                        bc[:, 0:672],
                        mybir.ActivationFunctionType.Abs,
                        bias=iota_sb[:, t : t + 1],
                        scale=-1.0,
                    )
                    nc.vector.tensor_scalar(
                        out=tent[:, 2 * t : 2 * t + 2, :].rearrange("p a b -> p (a b)"),
                        in0=dtmp[:],
                        scalar1=1.0,
                        scalar2=0.0,
                        op0=mybir.AluOpType.subtract,
                        op1=mybir.AluOpType.min,
                    )
                dtmp = dtmp_pool.tile([128, 672], F32, tag="dtmp")
                nc.scalar.activation(
                    dtmp[:, 0:336],
                    bc[:, 336:672],
                    mybir.ActivationFunctionType.Abs,
                    bias=iota_sb[:, 3:4],
                    scale=-1.0,
                )
                nc.vector.tensor_scalar(
                    out=tent[:, 6, :],
                    in0=dtmp[:, 0:336],
                    scalar1=1.0,
                    scalar2=0.0,
                    op0=mybir.AluOpType.subtract,
                    op1=mybir.AluOpType.min,
                )

                # pad fill: 127 - 127*vy_i*vx_j == vx*(-127*vy) + 127
                fill = fill_pool.tile([112, 3, 336], F32, tag="fill")
                for ic in range(3):
                    nc.vector.tensor_scalar(
                        out=fill[:, ic, :],
                        in0=bc[0:112, 672:1008],
                        scalar1=parcol_sb[0:112, b * 3 + ic : b * 3 + ic + 1],
                        scalar2=FILL,
                        op0=mybir.AluOpType.mult,
                        op1=mybir.AluOpType.add,
                    )

                # gather crop rows (slot 3c+t holds crop rows 128t..128t+127)
                crop = crop_pool.tile([128, 9, KCOLS], F32R, tag="crop")
                for s in range(9):
                    col = b * 9 + s
                    nc.gpsimd.indirect_dma_start(
                        out=crop[:, s, :],
                        out_offset=None,
                        in_=imgs[:, :],
                        in_offset=IndirectOffsetOnAxis(
                            ap=offs_sb[:, col : col + 1], axis=0
                        ),
                        bounds_check=BOUND,
                        oob_is_err=False,
                    )

                out_sb = out_pool.tile([112, 9, 336], F32, tag="osb")
                for c in range(C):
                    rt = rt_pool.tile([128, 4, 336], F32R, tag="rt")
                    for k2 in range(4):
                        pmm = ps1.tile([128, 336], F32, tag="pmm")
                        for t in range(3):
                            nc.tensor.matmul(
                                pmm[:],
                                crop[:, 3 * c + t, 128 * k2 : 128 * (k2 + 1)],
                                tent[:, 2 * t, :],
                                start=(t == 0),
                                stop=(t == 2),
                            )
                        nc.scalar.copy(rt[:, k2, :], pmm[:])
                    for ic in range(3):
                        pm2 = ps2.tile([112, 336], F32, tag="pm2")
                        for k2 in range(4):
                            nc.tensor.matmul(
                                pm2[:],
                                rt[:, k2, 112 * ic : 112 * (ic + 1)],
                                tent[:, 2 * k2 + 1 if k2 < 3 else 6, :],
                                start=(k2 == 0),
                                stop=(k2 == 3),
                            )
                        nc.vector.tensor_tensor(
                            out=out_sb[:, 3 * c + ic, :],
                            in0=pm2[:],
                            in1=fill[:, ic, :],
                            op=mybir.AluOpType.add,
                        )

                # store [112, (c,ic), 336] -> out[b] ([3,336,336] row-major)
                dst = AP(
                    tensor=out,
                    offset=b * C * O * O,
                    ap=[[O, 112], [112 * O, 9], [1, O]],
                )
                nc.sync.dma_start(dst, out_sb[:])

    nc.compile()
    return nc


def _host_params(images, boxes):
    """Per-core host prep. images: [BL,3,768,768] f32, boxes: [BL,4] i32."""
    f32 = np.float32
    offs = np.full((128, BL * 9), BIG, np.int32)
    par_rows = np.empty((1, BL * 1008), np.float32)  # broadcast at end
    par_cols = np.zeros((128, BL * 3), np.float32)

    grid = np.arange(O, dtype=np.int64)
    for b in range(BL):
        xb, yb, wb, hb = (int(v) for v in boxes[b])
        wf, hf = f32(wb), f32(hb)
        scale = f32(O) / np.maximum(wf, hf)
        new_w = int(np.round(wf * scale))
        new_h = int(np.round(hf * scale))
        pad_top = (O - new_h) // 2 if hb < wb else 0
        pad_left = (O - new_w) // 2 if hb >= wb else 0

        def axis_params(pad, new_n, nf, lim):
            i = grid - pad
            valid = (i >= 0) & (i < new_n)
            src = (i.astype(f32) + f32(0.5)) * nf
            src = src / f32(new_n)
            src = src - f32(0.5)        # crop-local source coordinate
            src = np.clip(src, f32(0.0), f32(lim - 1))
            src[~valid] = f32(-1e6)
            return src.astype(np.float32), valid.astype(np.float32)

        syc, vy = axis_params(pad_top, new_h, hf, hb)
        sxc, vx = axis_params(pad_left, new_w, wf, wb)
        # shift column coords by the xb%128 residual of the gather window
        x_shift = f32(xb - BLK * (xb // BLK))
        sxc = np.where(sxc > f32(-1e5), sxc + x_shift, sxc).astype(np.float32)

        par_rows[0, b * 1008 : b * 1008 + 336] = syc
        par_rows[0, b * 1008 + 336 : b * 1008 + 672] = sxc
        par_rows[0, b * 1008 + 672 : b * 1008 + 1008] = vx
        for ic in range(3):
            par_cols[0:112, b * 3 + ic] = -FILL * vy[ic * 112 : (ic + 1) * 112]

        # gather offsets (128-elem block indices): slot s = 3c+t,
        # partition p -> crop row 128t+p
        p = np.arange(128)
        xblk = xb // BLK
        for c in range(C):
            for t in range(3):
                r = 128 * t + p
                rr = np.minimum(r, hb - 1)
                off = (b * IMG_ELEMS + c * PLANE) // BLK + (yb + rr) * ROW_BLKS + xblk
                if b >= CROP_BUFS:
                    off = np.where(r < hb, off, BIG)
                offs[:, b * 9 + 3 * c + t] = off.astype(np.int32)

    iota = (np.arange(128)[:, None] + 128 * np.arange(4)[None, :]).astype(np.float32)
    return dict(
        imgs=np.ascontiguousarray(images).reshape(N_BLOCKS, BLK),
        offs=offs,
        par_row=np.ascontiguousarray(np.broadcast_to(par_rows, (128, BL * 1008))),
        par_col=par_cols,
        iota=iota,
    )


def kernel(images: np.ndarray, boxes: np.ndarray) -> np.ndarray:
    global _CACHED, LAST_RESULT
    if _CACHED is None:
        _CACHED = _build()
    nc = _CACHED

    in_maps = [
        _host_params(
            np.asarray(images[m * BL : (m + 1) * BL], dtype=np.float32),
            np.asarray(boxes[m * BL : (m + 1) * BL]),
        )
        for m in range(N_CORES)
    ]
    res = run_bass_kernel_spmd(nc, in_maps, core_ids=list(range(N_CORES)))
    LAST_RESULT = res
    return np.concatenate([r["out"] for r in res.results], axis=0)


# revision 6
# speedup vs baseline: 1.0612x; 1.0612x over previous
"""BoxCrop kernel for Trainium2 (8 NeuronCores, Bass/Tile).

Fused crop -> aspect-preserving bilinear resize (long side 336) -> square pad
(fill=127) for a batch of 64 images [64,3,768,768] with per-image XYWH boxes.

Strategy (pure data-parallel, 8 images per core):
- Host computes, per image: crop-row gather offsets (128-element-block
  indices into the image shard viewed as [n_blocks, 128]; each descriptor
  reads 512 contiguous elements = 4 blocks covering image columns
  [128*(xb//128), +512) which always contains the crop columns), clamped
  crop-local source coordinates (syc for rows; sxc shifted by the xb%128
  residual for columns; -1e6 for pad rows/cols), and pad-fill params.
- Device per image-channel:
    crop[r, 0:512] via indirect-DMA row gather (f32r, 512B+ descriptors);
    A'[r,i]  = -relu(1-|r-syc_i|)   r in [0,384)   (3 chunks)
    Wx'[k,j] = -relu(1-|k-sxc_j|)   k in [0,512)   (4 chunks)
    (negated tents == exact bilinear weights incl. boundary clamping;
     the two negations cancel in the product)
    RT = crop^T @ A'  (f32r matmuls, contraction over r, partition = k);
    M  = RT-contraction with Wx' (over k) accumulated in PSUM;
    out = M + (127 - 127*vy_i*vx_j)  (pad fill) during PSUM->SBUF.
"""
import numpy as np

import concourse.bacc as bacc
import concourse.bass as bass
import concourse.tile as tile
from concourse import mybir
from concourse.bass import AP, IndirectOffsetOnAxis
from concourse.bass_utils import run_bass_kernel_spmd

F32 = mybir.dt.float32
F32R = mybir.dt.float32r
I32 = mybir.dt.int32

N_CORES = 8
B = 64
BL = B // N_CORES          # images per core
C = 3
H = W = 768
O = 336                    # output size
RROWS = 384                # gathered crop rows (static max)
KCOLS = 512                # gathered columns per row (4 x 128-elem blocks)
PLANE = H * W
IMG_ELEMS = C * PLANE
TOT_ELEMS = BL * IMG_ELEMS
BLK = 128
N_BLOCKS = TOT_ELEMS // BLK
ROW_BLKS = W // BLK        # 6 blocks per image row
BOUND = N_BLOCKS - KCOLS // BLK   # max valid gather start block
BIG = 1 << 27
FILL = 127.0
CROP_BUFS = 3              # first CROP_BUFS images must fully write their tile

_CACHED = None
LAST_RESULT = None


def _build(reps: int = 1):
    nc = bacc.Bacc("TRN2", target_bir_lowering=False, debug=False)

    imgs = nc.dram_tensor("imgs", [N_BLOCKS, BLK], F32R, kind="ExternalInput")
    offs = nc.dram_tensor("offs", [128, BL * 9], I32, kind="ExternalInput")
    par_row = nc.dram_tensor("par_row", [128, BL * 672], F32, kind="ExternalInput")
    par_col = nc.dram_tensor("par_col", [128, BL * 3], F32, kind="ExternalInput")
    iota = nc.dram_tensor("iota", [128, 4], F32, kind="ExternalInput")
    out = nc.dram_tensor("out", [BL, C, O, O], F32, kind="ExternalOutput")

    with tile.TileContext(nc) as tc:
        with (
            tc.tile_pool(name="const", bufs=1) as cpool,
            tc.tile_pool(name="crop", bufs=CROP_BUFS) as crop_pool,
            tc.tile_pool(name="tent", bufs=2) as tent_pool,
            tc.tile_pool(name="dtmp", bufs=3) as dtmp_pool,
            tc.tile_pool(name="fill", bufs=2) as fill_pool,
            tc.tile_pool(name="rt", bufs=3) as rt_pool,
            tc.tile_pool(name="osb", bufs=2) as out_pool,
            tc.tile_pool(name="ps1", bufs=4, space="PSUM") as ps1,
            tc.tile_pool(name="ps2", bufs=4, space="PSUM") as ps2,
        ):
            offs_sb = cpool.tile([128, BL * 9], I32, tag="offs")
            nc.sync.dma_start(offs_sb[:], offs[:])
            par_sb = cpool.tile([128, BL * 672], F32, tag="par")
            nc.sync.dma_start(par_sb[:], par_row[:])
            parcol_sb = cpool.tile([128, BL * 3], F32, tag="parcol")
            nc.sync.dma_start(parcol_sb[:], par_col[:])
            iota_sb = cpool.tile([128, 4], F32, tag="iota")
            nc.sync.dma_start(iota_sb[:], iota[:])

            for b in range(reps * BL):
                b = b % BL
                bc = par_sb[:, b * 672 : (b + 1) * 672]

                # negated tents, f32r. tent slots: 2t   = A' chunk t (t<3)
                #                                  2t+1 = Wx' chunk t (t<3)
                #                                  6    = Wx' chunk 3
                tent = tent_pool.tile([128, 7, 336], F32R, tag="tent")
                for t in range(3):
                    dtmp = dtmp_pool.tile([128, 672], F32, tag="dtmp")
                    nc.scalar.activation(
                        dtmp[:],
                        bc[:, 0:672],
                        mybir.ActivationFunctionType.Abs,
                        bias=iota_sb[:, t : t + 1],
                        scale=-1.0,
                    )
                    nc.vector.tensor_scalar(
                        out=tent[:, 2 * t : 2 * t + 2, :].rearrange("p a b -> p (a b)"),
                        in0=dtmp[:],
                        scalar1=1.0,
                        scalar2=0.0,
                        op0=mybir.AluOpType.subtract,
                        op1=mybir.AluOpType.min,
                    )
                dtmp = dtmp_pool.tile([128, 672], F32, tag="dtmp")
                nc.scalar.activation(
                    dtmp[:, 0:336],
                    bc[:, 336:672],
                    mybir.ActivationFunctionType.Abs,
                    bias=iota_sb[:, 3:4],
                    scale=-1.0,
                )
                nc.vector.tensor_scalar(
                    out=tent[:, 6, :],
                    in0=dtmp[:, 0:336],
                    scalar1=1.0,
                    scalar2=0.0,
                    op0=mybir.AluOpType.subtract,
                    op1=mybir.AluOpType.min,
                )

                # pad fill: 127 - 127*vy_i*vx_j == vx*(-127*vy) + 127
                # vx reconstructed on device: vx = (sxc >= -1e5)
                fill = fill_pool.tile([112, 4, 336], F32, tag="fill")
                nc.vector.tensor_scalar(
                    out=fill[:, 3, :],
                    in0=bc[0:112, 336:672],
                    scalar1=-1e5,
                    scalar2=None,
                    op0=mybir.AluOpType.is_ge,
                )
                for ic in range(3):
                    nc.vector.tensor_scalar(
                        out=fill[:, ic, :],
                        in0=fill[:, 3, :],
                        scalar1=parcol_sb[0:112, b * 3 + ic : b * 3 + ic + 1],
                        scalar2=FILL,
                        op0=mybir.AluOpType.mult,
                        op1=mybir.AluOpType.add,
                    )

                # gather crop rows (slot 3c+t holds crop rows 128t..128t+127)
                crop = crop_pool.tile([128, 9, KCOLS], F32R, tag="crop")
                for s in range(9):
                    col = b * 9 + s
                    nc.gpsimd.indirect_dma_start(
                        out=crop[:, s, :],
                        out_offset=None,
                        in_=imgs[:, :],
                        in_offset=IndirectOffsetOnAxis(
                            ap=offs_sb[:, col : col + 1], axis=0
                        ),
                        bounds_check=BOUND,
                        oob_is_err=False,
                    )

                out_sb = out_pool.tile([112, 9, 336], F32, tag="osb")
                for c in range(C):
                    rt = rt_pool.tile([128, 4, 336], F32R, tag="rt")
                    for k2 in range(4):
                        pmm = ps1.tile([128, 336], F32, tag="pmm")
                        for t in range(3):
                            nc.tensor.matmul(
                                pmm[:],
                                crop[:, 3 * c + t, 128 * k2 : 128 * (k2 + 1)],
                                tent[:, 2 * t, :],
                                start=(t == 0),
                                stop=(t == 2),
                            )
                        nc.scalar.copy(rt[:, k2, :], pmm[:])
                    for ic in range(3):
                        pm2 = ps2.tile([112, 336], F32, tag="pm2")
                        for k2 in range(4):
                            nc.tensor.matmul(
                                pm2[:],
                                rt[:, k2, 112 * ic : 112 * (ic + 1)],
                                tent[:, 2 * k2 + 1 if k2 < 3 else 6, :],
                                start=(k2 == 0),
                                stop=(k2 == 3),
                            )
                        nc.vector.tensor_tensor(
                            out=out_sb[:, 3 * c + ic, :],
                            in0=pm2[:],
                            in1=fill[:, ic, :],
                            op=mybir.AluOpType.add,
                        )

                    # store channel c: [112, 3, 336] -> out[b, c]
                    dst = AP(
                        tensor=out,
                        offset=(b * C + c) * O * O,
                        ap=[[O, 112], [112 * O, 3], [1, O]],
                    )
                    nc.sync.dma_start(dst, out_sb[:, 3 * c : 3 * c + 3, :])

    nc.compile()
    return nc


def _host_params(images, boxes):
    """Per-core host prep. images: [BL,3,768,768] f32, boxes: [BL,4] i32."""
    f32 = np.float32
    offs = np.full((128, BL * 9), BIG, np.int32)
    par_rows = np.empty((1, BL * 672), np.float32)  # broadcast at end
    par_cols = np.zeros((128, BL * 3), np.float32)

    grid = np.arange(O, dtype=np.int64)
    for b in range(BL):
        xb, yb, wb, hb = (int(v) for v in boxes[b])
        wf, hf = f32(wb), f32(hb)
        scale = f32(O) / np.maximum(wf, hf)
        new_w = int(np.round(wf * scale))
        new_h = int(np.round(hf * scale))
        pad_top = (O - new_h) // 2 if hb < wb else 0
        pad_left = (O - new_w) // 2 if hb >= wb else 0

        def axis_params(pad, new_n, nf, lim):
            i = grid - pad
            valid = (i >= 0) & (i < new_n)
            src = (i.astype(f32) + f32(0.5)) * nf
            src = src / f32(new_n)
            src = src - f32(0.5)        # crop-local source coordinate
            src = np.clip(src, f32(0.0), f32(lim - 1))
            src[~valid] = f32(-1e6)
            return src.astype(np.float32), valid.astype(np.float32)

        syc, vy = axis_params(pad_top, new_h, hf, hb)
        sxc, vx = axis_params(pad_left, new_w, wf, wb)
        # shift column coords by the xb%128 residual of the gather window
        x_shift = f32(xb - BLK * (xb // BLK))
        sxc = np.where(sxc > f32(-1e5), sxc + x_shift, sxc).astype(np.float32)

        par_rows[0, b * 672 : b * 672 + 336] = syc
        par_rows[0, b * 672 + 336 : b * 672 + 672] = sxc
        for ic in range(3):
            par_cols[0:112, b * 3 + ic] = -FILL * vy[ic * 112 : (ic + 1) * 112]

        # gather offsets (128-elem block indices): slot s = 3c+t,
        # partition p -> crop row 128t+p
        p = np.arange(128)
        xblk = xb // BLK
        for c in range(C):
            for t in range(3):
                r = 128 * t + p
                rr = np.minimum(r, hb - 1)
                off = (b * IMG_ELEMS + c * PLANE) // BLK + (yb + rr) * ROW_BLKS + xblk
                if b >= CROP_BUFS:
                    off = np.where(r < hb, off, BIG)
                offs[:, b * 9 + 3 * c + t] = off.astype(np.int32)

    iota = (np.arange(128)[:, None] + 128 * np.arange(4)[None, :]).astype(np.float32)
    return dict(
        imgs=np.ascontiguousarray(images).reshape(N_BLOCKS, BLK),
        offs=offs,
        par_row=np.ascontiguousarray(np.broadcast_to(par_rows, (128, BL * 672))),
        par_col=par_cols,
        iota=iota,
    )


def kernel(images: np.ndarray, boxes: np.ndarray) -> np.ndarray:
    global _CACHED, LAST_RESULT
    if _CACHED is None:
        _CACHED = _build()
    nc = _CACHED

    in_maps = [
        _host_params(
            np.asarray(images[m * BL : (m + 1) * BL], dtype=np.float32),
            np.asarray(boxes[m * BL : (m + 1) * BL]),
        )
        for m in range(N_CORES)
    ]
    res = run_bass_kernel_spmd(nc, in_maps, core_ids=list(range(N_CORES)))
    LAST_RESULT = res
    return np.concatenate([r["out"] for r in res.results], axis=0)
